# revision 38
# baseline (speedup 1.0000x reference)
"""Dual-stream attention (nn_Attention2) on 8 TRN2 NeuronCores, v3.

Problem: B=4, N=1024, C=768, H=12, D=64.
  qkv_s = x_s @ W_qkv.T + b_qkv          (s = 1,2; shared weights)
  attn  = softmax(q1k1/sqrt(D) + q2k2/sqrt(D))   (one shared softmax)
  o_s   = attn @ v_s;  y_s = o_s @ W_proj.T + b_proj

Sharding: 8 cores = 4 batches x 2 head-groups (6 heads each). v3 drops
ALL on-device collectives: every core receives its full inputs (x for
its batch, weights for its head group) pre-staged in device DRAM in
SBUF-ready [128, free] layout, so SBUF loads are single contiguous DMAs
and the PE starts ~4us in (v2 burned ~30us on staging copies +
AllGathers). Each core emits its PARTIAL projection output (both
streams, bf16); the host adds the pair partials (the "unshard" step),
which replaces v2's tail ReduceScatter.

Bias handling: q/k biases are applied for free on the mandatory
psum->SBUF drains (ACT Identity-with-bias / DVE tensor_scalar).  v and
proj biases fold into a single host-side constant vector: softmax rows
sum to 1, so  o = attn@(v + bv) = attn@v + bv  and the bias lands in
y as  W_proj @ bv + b_proj, added on the host.

Compute phases (per core, stream-stacked [s1 64 | s2 64] q/k tiles so
combined scores are one matmul chain; attention in sT[k,q] orientation;
ones-matmul gives the softmax denominator broadcast across partitions):
  1. qkv: 12 q/k passes ([128,1024] psum, drains alternate DVE/ACT),
     16 v passes ([128,384] psum, plain-copy drains).
  2. attention per head: scores 2 tiles ahead, exp on ACT (bf16 out),
     AV accumulated into a single [128,1024] psum, bf16 pairwise
     add-tree on DVE + ones-matmul denominator in its OWN psum tag
     (v2 aliased it with the scores ring, serializing ~2us/head),
     reciprocal+normalize on DVE, finalize deferred one head.
  3. projection: per (q,cb,s) 3-matmul chains, Copy drains on ACT/DVE,
     partial y DMAed straight out (no collective, no device bias).
bf16 matmuls, f32 PSUM.
"""

import contextlib
import threading

import numpy as np
import ml_dtypes
import jax
from jax.sharding import Mesh, PartitionSpec
try:
    from jax.experimental.shard_map import shard_map
except ImportError:
    from jax.sharding import shard_map

import concourse.bass as bass
import concourse.tile as tile
from concourse import bacc, mybir
from concourse.bass_utils import run_bass_kernel_spmd
from concourse.bass2jax import (
    install_neuronx_cc_hook,
    partition_id_tensor,
    _bass_exec_p,
)

F32 = mybir.dt.float32
BF16 = mybir.dt.bfloat16
AL = mybir.AluOpType
AF = mybir.ActivationFunctionType

B, N, C, H = 4, 1024, 768, 12
D = C // H              # 64
HPC = 6                 # heads per core
KT = C // 128           # 6 contraction tiles over C
NQ = N // 512           # 2 q-halves
NK = N // 128           # 8 k-blocks
SCALE = float(D) ** -0.5
BF = ml_dtypes.bfloat16


def build_program(loop_reps=0, phase_cut=None, use_cc=True):
    """use_cc is accepted for test.py compat; v3 has no collectives, so
    the timing build and the graded build are the same program."""
    del use_cc
    nc = bacc.Bacc("TRN2", target_bir_lowering=False, debug=False)

    # inputs pre-staged in DRAM in SBUF layout ([128, free], bf16)
    wqk_d = nc.dram_tensor("wqk", [128, KT * C], BF16, kind="ExternalInput").ap()
    wv_d = nc.dram_tensor("wv", [128, KT * HPC * D], BF16,
                          kind="ExternalInput").ap()
    wp_d = nc.dram_tensor("wp", [128, (HPC // 2) * C], BF16,
                          kind="ExternalInput").ap()
    x_d = [nc.dram_tensor(f"x{s}", [128, KT * N], BF16,
                          kind="ExternalInput").ap() for s in range(2)]
    bq_d = nc.dram_tensor("bq", [128, HPC], F32, kind="ExternalInput").ap()
    yp = nc.dram_tensor("yp", [2 * C, N], BF16, kind="ExternalOutput").ap()

    with tile.TileContext(nc) as tc:
        with (
            tc.tile_pool(name="persist", bufs=1) as pp,
            tc.tile_pool(name="expp", bufs=16) as ep,
            tc.tile_pool(name="rdp", bufs=2) as rp,
            tc.tile_pool(name="ybp", bufs=4) as yp_pool,
            tc.For_i(0, loop_reps, 1) if loop_reps else contextlib.nullcontext(),
        ):
            # ---- SBUF loads: contiguous DMAs straight from DRAM inputs ----
            wqk_sb = pp.tile([128, KT * C], BF16, tag="wqk", name="wqk")
            x_sb = [pp.tile([128, KT * N], BF16, tag=f"x{s}", name=f"x{s}")
                    for s in range(2)]
            wv_sb = pp.tile([128, KT * HPC * D], BF16, tag="wv", name="wv")
            wp_sb = pp.tile([128, (HPC // 2) * C], BF16, tag="wp", name="wp")
            bq_sb = pp.tile([128, HPC], F32, tag="bq")
            # The DMA fabric is effectively one serial ~360GB/s resource and
            # cross-queue arbitration scrambles ordering, so issue ALL input
            # loads on one queue in exactly the order compute consumes them.
            half = KT * 512
            for k in range(KT):
                nc.sync.dma_start(out=wqk_sb[:, k * C:(k + 1) * C],
                                  in_=wqk_d[:, k * C:(k + 1) * C])
            nc.sync.dma_start(out=x_sb[0][:, 0:half], in_=x_d[0][:, 0:half])
            nc.sync.dma_start(out=bq_sb, in_=bq_d)
            nc.sync.dma_start(out=x_sb[0][:, half:2 * half],
                              in_=x_d[0][:, half:2 * half])
            nc.sync.dma_start(out=x_sb[1][:, 0:half], in_=x_d[1][:, 0:half])
            nc.sync.dma_start(out=x_sb[1][:, half:2 * half],
                              in_=x_d[1][:, half:2 * half])
            nc.sync.dma_start(out=wv_sb, in_=wv_d)
            nc.sync.dma_start(out=wp_sb, in_=wp_d)
            ones = pp.tile([128, 128], BF16, tag="ones")
            nc.vector.memset(ones, 1.0)
            # PE p-state warmup: the tensor engine needs ~3us of continuous
            # execution to ramp 0.65 -> 2.4 GHz, and any idle resets it.
            # Dummy matmuls from t~0.5us keep it pinned hot until the first
            # x chunk lands (~9us), so real passes start at full clock.
            warm = pp.tile([128, 512], BF16, tag="warm")
            nc.vector.memset(warm, 0.0)

            # x_sb column layout: q*(KT*512) + k*512 + t  (t in 0..511)
            def xcol(s, q, k, t0, n):
                base = q * (KT * 512) + k * 512 + t0
                return x_sb[s][:, base:base + n]

            # ---- phase 1: q/k then v projections ----
            qt = [pp.tile([128, N], BF16, tag=f"qt{h}", name=f"qt{h}")
                  for h in range(HPC)]
            kt_ = [pp.tile([128, N], BF16, tag=f"kt{h}", name=f"kt{h}")
                   for h in range(HPC)]
            vt = [pp.tile([128, HPC * 128], BF16, tag=f"vt{t}", name=f"vt{t}")
                  for t in range(NK)]

            # ps_s is allocated FIRST so its banks never alias the qkv
            # pools' -- head 0/1 score tiles must not wait on qk drains.
            ps_s = tc.alloc_tile_pool(name="ps_s", bufs=2, space="PSUM")
            ps_qk = tc.alloc_tile_pool(name="ps_qk", bufs=4, space="PSUM")

            def qk_pass(ft, s, q):
                # one q-half per pass: [128,512] psum (1 bank), 6 matmuls
                p = ps_qk.tile([128, 512], F32, tag="qkp", name="qkp")
                for k in range(KT):
                    nc.tensor.matmul(
                        p,
                        lhsT=wqk_sb[:, k * C + ft * 128:
                                    k * C + (ft + 1) * 128],
                        rhs=xcol(s, q, k, 0, 512),
                        start=(k == 0), stop=(k == KT - 1))
                pair = qt if ft < HPC // 2 else kt_
                h0 = (ft % (HPC // 2)) * 2
                ql = slice(q * 512, (q + 1) * 512)
                # one drain per head-half: hf=0 on DVE, hf=1 on ACT
                nc.vector.tensor_scalar(
                    out=pair[h0][s * 64:(s + 1) * 64, ql],
                    in0=p[0:64, :],
                    scalar1=bq_sb[0:64, ft:ft + 1],
                    scalar2=None, op0=AL.add)
                nc.scalar.activation(
                    out=pair[h0 + 1][s * 64:(s + 1) * 64, ql],
                    in_=p[64:128, :],
                    func=AF.Identity,
                    bias=bq_sb[64:128, ft:ft + 1])

            def v_pass(s, t):
                p = ps_v.tile([128, HPC * D], F32, tag="vp", name="vp")
                q = t // 4
                t0 = t * 128 - q * 512
                for k in range(KT):
                    nc.tensor.matmul(
                        p,
                        lhsT=xcol(s, q, k, t0, 128),
                        rhs=wv_sb[:, k * HPC * D:(k + 1) * HPC * D],
                        start=(k == 0), stop=(k == KT - 1))
                out3 = vt[t].rearrange(
                    "p (h two d) -> p h two d", two=2, d=D)[:, :, s, :]
                src = p.rearrange("p (h d) -> p h d", d=D)
                # all v drains on DVE: ACT is running head-0/1 exps by now
                nc.vector.tensor_copy(out=out3, in_=src)

            # (s, q) outer so stream 0's passes only need x0 (arrives
            # first); ft ordered so head 0/1's q AND k finish earliest --
            # the scheduler then starts head-0 scores/exp during the
            # qkv tail. Live psum banks: qk phase sp(4)+qk(4)=8; v phase
            # sp(4)+op2(2)+v(2)=8; ps_d reuses v's banks after release.
            for i in range(36):
                pw = ps_qk.tile([128, 512], F32, tag="qkp", name="warm")
                nc.tensor.matmul(pw, lhsT=ones, rhs=warm,
                                 start=True, stop=True)
            for s in range(2):
                for q in range(NQ):
                    for ft in (0, 3, 1, 4, 2, 5):
                        qk_pass(ft, s, q)
            ps_qk.release()
            dsp = tc.alloc_tile_pool(name="dsp", bufs=2)
            ps_o = tc.alloc_tile_pool(name="ps_o", bufs=1, space="PSUM")
            ps_v = tc.alloc_tile_pool(name="ps_v", bufs=2, space="PSUM")
            # v passes for t>=2 are woven into head 0's loop below (PE
            # filler that keeps the tensor engine warm while the exp
            # stream paces); the debug cut emits them all here instead.
            if phase_cut == "qkv":
                for t in range(NK):
                    for s in range(2):
                        v_pass(s, t)
                ps_v.release()
                for h in range(HPC):
                    nc.sync.dma_start(out=yp[h * 128:(h + 1) * 128, 0:512],
                                      in_=qt[h][:, 0:512])
                    nc.sync.dma_start(out=yp[h * 128:(h + 1) * 128, 512:1024],
                                      in_=kt_[h][:, 0:512])
                for t in range(NK):
                    nc.sync.dma_start(
                        out=yp[C + (t % 6) * 128:C + (t % 6 + 1) * 128,
                               (t // 6) * 128:(t // 6) * 128 + 128],
                        in_=vt[t][:, 0:128])

            if phase_cut in (None, "attn"):
                # ---- phase 2: attention per head, sT[k, q] orientation ----
                # The exp stream on ACT is the pacer (8 x ~1.04us per head);
                # everything else is scheduled to never starve it:
                #  * scores for kb+2 are emitted ahead of all other PE work
                #    in each iteration (2-deep psum prefetch ring),
                #  * AV accumulation is rotated to start at kb5 and the
                #    kb0..4 products are deferred into the NEXT head's first
                #    two iterations, so the op2 psum buffer (single, PSUM is
                #    full) is reused only after the previous head's
                #    normalize has cleared -- no boundary stall,
                #  * the denominator uses a running sum (one DVE add after
                #    the last exp instead of a 3-level tree tail) and the
                #    finalize chain (ones-matmul -> reciprocal -> normalize,
                #    q-half split) is emitted at kb1 of the next head.
                ost = [[pp.tile([128, N], BF16, tag=f"ost{s}_{p}",
                                name=f"ost{s}_{p}")
                        for p in range(HPC // 2)] for s in range(2)]
                ps_d = None  # allocated at the h==1 boundary (reuses ps_v)

                AVS = 5  # first kb of the in-head AV accumulation window

                def scores_tile(h, kb, use_dp=False):
                    # use_dp: time-share the denominator psum banks for the
                    # next head's kb0 tile -- a 3rd effective prefetch slot
                    # that removes the head-boundary WAR stall on sp0.
                    if use_dp:
                        spt = ps_d.tile([128, N], F32, tag="dp",
                                        name="sp", bufs=1)
                    else:
                        spt = ps_s.tile([128, N], F32, tag=f"sp{kb % 2}",
                                        name="sp", bufs=1)
                    for q in range(NQ):
                        nc.tensor.matmul(
                            spt[:, q * 512:(q + 1) * 512],
                            lhsT=kt_[h][:, kb * 128:(kb + 1) * 128],
                            rhs=qt[h][:, q * 512:(q + 1) * 512],
                            start=True, stop=True)
                    return spt

                def av(h, kb, ex_t, op2_t):
                    for q in range(NQ):
                        nc.tensor.matmul(
                            op2_t[:, q * 512:(q + 1) * 512],
                            lhsT=vt[kb][:, h * 128:(h + 1) * 128],
                            rhs=ex_t[:, q * 512:(q + 1) * 512],
                            start=(kb == AVS), stop=(kb == AVS - 1))

                def fin_emit(h, st):
                    dp2 = ps_d.tile([128, N], F32, tag="dp",
                                    name="dp2", bufs=1)
                    rd = rp.tile([128, N], F32, tag="rd", name="rd")
                    for q in range(NQ):
                        ql = slice(q * 512, (q + 1) * 512)
                        nc.tensor.matmul(
                            dp2[:, ql], lhsT=ones, rhs=st["es"][:, ql],
                            start=True, stop=True)
                        nc.vector.reciprocal_approx_fast(
                            out=rd[:, ql], in_=dp2[:, ql])
                        for s in range(2):
                            nc.vector.tensor_mul(
                                out=ost[s][h // 2][(h % 2) * 64:
                                                   (h % 2) * 64 + 64, ql],
                                in0=st["op2"][s * 64:(s + 1) * 64, ql],
                                in1=rd[s * 64:(s + 1) * 64, ql])

                prev = None
                sp01 = [scores_tile(0, 0), scores_tile(0, 1)]
                for t in (0, 1):
                    for s in range(2):
                        v_pass(s, t)
                for h in range(HPC):
                    if h == 1:
                        # all v psum work was emitted in head 0's loop
                        ps_v.release()
                        ps_d = tc.alloc_tile_pool(name="ps_d", bufs=1,
                                                  space="PSUM")
                    op2 = ps_o.tile([128, N], F32, tag="op2", name="op2")
                    sp = [None] * NK
                    sp[0], sp[1] = sp01
                    ex = [None] * NK
                    run = None
                    es = None
                    for kb in range(NK):
                        ex[kb] = ep.tile([128, N], BF16, tag="exp", name="exp")
                        nc.scalar.activation(out=ex[kb], in_=sp[kb],
                                             func=AF.Exp)
                        if kb + 2 < NK:
                            sp[kb + 2] = scores_tile(h, kb + 2)
                        if h == 0 and kb + 2 < NK:
                            # weave the remaining v passes into head 0
                            v_pass(0, kb + 2)
                            v_pass(1, kb + 2)
                        if prev is not None:
                            if kb == 0:
                                for dkb in range(3):
                                    av(h - 1, dkb, prev["ex"][dkb],
                                       prev["op2"])
                            elif kb == 1:
                                for dkb in range(3, AVS):
                                    av(h - 1, dkb, prev["ex"][dkb],
                                       prev["op2"])
                                fin_emit(h - 1, prev)
                        if h + 1 < HPC:
                            # pre-emit next head's first score tiles ahead
                            # of this head's last AVs so exp(h+1,0) starts
                            # with zero bubble at the boundary; from h>=1
                            # kb0 goes through the dp banks (free until the
                            # next fin), which unpins the sp0 WAR entirely
                            if h >= 1 and kb == AVS:
                                sp01 = [scores_tile(h + 1, 0, use_dp=True),
                                        None]
                            elif h == 0 and kb == NK - 2:
                                sp01 = [scores_tile(h + 1, 0), None]
                            elif kb == NK - 1:
                                sp01[1] = scores_tile(h + 1, 1)
                        if kb >= AVS:
                            av(h, kb, ex[kb], op2)
                            if h == HPC - 1:
                                # last head: fold the deferred AVs into the
                                # in-head window so finalize fires at once
                                for dkb in range(2 * (kb - AVS),
                                                 min(2 * (kb - AVS) + 2, AVS)):
                                    av(h, dkb, ex[dkb], op2)
                        # running-sum denominator on DVE
                        if kb == 1:
                            run = dsp.tile([128, N], BF16, tag="run",
                                           name="run")
                            nc.vector.tensor_tensor(out=run, in0=ex[0],
                                                    in1=ex[1], op=AL.add)
                        elif 2 <= kb <= NK - 2:
                            nrun = dsp.tile([128, N], BF16, tag="run",
                                            name="run")
                            nc.vector.tensor_tensor(out=nrun, in0=run,
                                                    in1=ex[kb], op=AL.add)
                            run = nrun
                        else:
                            if kb == NK - 1:
                                es = dsp.tile([128, N], BF16, tag="es",
                                              name="es")
                                nc.vector.tensor_tensor(out=es, in0=run,
                                                        in1=ex[kb], op=AL.add)
                    prev = {"op2": op2, "ex": ex, "es": es}
                # last head's AVs were folded in-loop; just finalize
                fin_emit(HPC - 1, prev)
                ps_d.release()
                ps_o.release()
                dsp.release()
                ps_s.release()

                if phase_cut == "attn":
                    for s in range(2):
                        for p_ in range(HPC // 2):
                            nc.sync.dma_start(
                                out=yp[(s * 3 + p_) * 128:
                                       (s * 3 + p_ + 1) * 128, :],
                                in_=ost[s][p_])

            if phase_cut is None:
                # ---- phase 3: projection; partial y straight to DRAM ----
                # (cb,s) outer / q inner so both q-halves land in one yb
                # tile -> 12 full-row DMAs, split across two queues so
                # descriptor generation doesn't pace the tail.
                ps_y = tc.alloc_tile_pool(name="ps_y", bufs=6, space="PSUM")
                NP = HPC // 2
                # bridge the last finalize chain (~3.5us of serial DVE work)
                # with dummy matmuls so the PE clock stays hot into the proj
                for i in range(14):
                    pw = ps_y.tile([128, 512], F32, tag="yp", name="warm")
                    nc.tensor.matmul(pw, lhsT=ones, rhs=warm,
                                     start=True, stop=True)
                for cb in range(C // 128):
                    for s in range(2):
                        yb = yp_pool.tile([128, N], BF16, tag="yb")
                        for q in range(NQ):
                            py = ps_y.tile([128, 512], F32, tag="yp",
                                           name="yp")
                            for p in range(NP):
                                nc.tensor.matmul(
                                    py,
                                    lhsT=wp_sb[:, p * C + cb * 128:
                                               p * C + (cb + 1) * 128],
                                    rhs=ost[s][p][:, q * 512:(q + 1) * 512],
                                    start=(p == 0), stop=(p == NP - 1))
                            if (q + s) % 2 == 0:
                                nc.scalar.activation(
                                    out=yb[:, q * 512:(q + 1) * 512],
                                    in_=py, func=AF.Copy)
                            else:
                                nc.vector.tensor_copy(
                                    out=yb[:, q * 512:(q + 1) * 512], in_=py)
                        eng = nc.sync if (cb + s) % 2 == 0 else nc.gpsimd
                        eng.dma_start(
                            out=yp[s * C + cb * 128:s * C + (cb + 1) * 128, :],
                            in_=yb)
                ps_y.release()

    nc.compile()
    return nc


_cache = threading.Lock()
_nc = None
_runner = None


def _get_program():
    global _nc
    with _cache:
        if _nc is None:
            _nc = build_program()
    return _nc


class _Runner:
    """Compile the 8-core sharded PJRT callable once and reuse it across
    kernel() calls (run_bass_kernel_spmd re-traces jax.jit per call, which
    costs seconds; the NEFF itself is what actually runs)."""

    def __init__(self, nc, n_cores=8):
        install_neuronx_cc_hook()
        self.nc = nc
        self.n_cores = n_cores
        partition_name = (nc.partition_id_tensor.name
                          if nc.partition_id_tensor else None)
        in_names, out_names, out_avals, zero_outs = [], [], [], []
        for alloc in nc.m.functions[0].allocations:
            if not isinstance(alloc, mybir.MemoryLocationSet):
                continue
            name = alloc.memorylocations[0].name
            if alloc.kind == "ExternalInput":
                if name != partition_name:
                    in_names.append(name)
            elif alloc.kind == "ExternalOutput":
                out_names.append(name)
                shape = tuple(alloc.tensor_shape)
                dtype = mybir.dt.np(alloc.dtype)
                out_avals.append(jax.core.ShapedArray(shape, dtype))
                zero_outs.append(
                    np.zeros((n_cores * shape[0], *shape[1:]), dtype))
        self.in_names = in_names
        self.out_names = out_names
        self.out_shapes = [tuple(a.shape) for a in out_avals]
        self.zero_outs = zero_outs
        n_params = len(in_names)
        n_outs = len(out_avals)
        all_in = list(in_names) + list(out_names)
        if partition_name is not None:
            all_in.append(partition_name)

        def _body(*args):
            operands = list(args)
            if partition_name is not None:
                operands.append(partition_id_tensor())
            outs = _bass_exec_p.bind(
                *operands,
                out_avals=tuple(out_avals),
                in_names=tuple(all_in),
                out_names=tuple(out_names),
                lowering_input_output_aliases=(),
                sim_require_finite=True,
                sim_require_nnan=True,
                nc=nc,
            )
            return tuple(outs)

        devices = jax.devices()[:n_cores]
        mesh = Mesh(np.asarray(devices), ("core",))
        self.f = jax.jit(
            shard_map(
                _body, mesh=mesh,
                in_specs=(PartitionSpec("core"),) * (n_params + n_outs),
                out_specs=(PartitionSpec("core"),) * n_outs,
                check_rep=False,
            ),
            keep_unused=True,
        )

    def run(self, in_maps):
        n = self.n_cores
        concat_in = [
            np.concatenate([np.asarray(in_maps[c][name]) for c in range(n)],
                           axis=0)
            for name in self.in_names
        ]
        out_arrs = self.f(*concat_in, *self.zero_outs)
        return [
            {name: np.asarray(out_arrs[i]).reshape(n, *self.out_shapes[i])[c]
             for i, name in enumerate(self.out_names)}
            for c in range(n)
        ]


def _get_runner():
    global _runner
    nc = _get_program()
    with _cache:
        if _runner is None:
            _runner = _Runner(nc)
    return _runner


def _f32_to_bf16(a):
    """Fast round-to-nearest f32->bf16 via integer ops (contiguous input)."""
    u = np.ascontiguousarray(a, np.float32).view(np.uint32)
    return (((u + 0x7FFF) + ((u >> 16) & 1)) >> 16).astype(np.uint16).view(BF)


def _bf16_to_f32(a):
    return (np.asarray(a).view(np.uint16).astype(np.uint32) << 16).view(
        np.float32)


_wprep_cache = {}


def _prep_weights(W_qkv, b_qkv, W_proj, b_proj):
    key = (id(W_qkv), id(b_qkv), id(W_proj), id(b_proj))
    hit = _wprep_cache.get(key)
    if hit is not None and (hit[0] is W_qkv and hit[1] is b_qkv
                            and hit[2] is W_proj and hit[3] is b_proj):
        return hit[4]
    W_qkv = np.asarray(W_qkv, np.float32)
    b_qkv = np.asarray(b_qkv, np.float32)
    W_proj = np.asarray(W_proj, np.float32)
    b_proj = np.asarray(b_proj, np.float32)
    Wq = W_qkv[0:C].reshape(H, D, C) * SCALE
    Wk = W_qkv[C:2 * C].reshape(H, D, C)
    Wv = W_qkv[2 * C:3 * C].reshape(H, D, C)
    bq = b_qkv[0:C].reshape(H, D) * SCALE
    bk = b_qkv[C:2 * C].reshape(H, D)
    bvv = b_qkv[2 * C:3 * C]

    per_group = []
    for g in range(2):
        hs = slice(g * HPC, (g + 1) * HPC)
        # wqk_sb[p, k*768 + ft*128 + hf*64 + d] =
        #   (Wq | Wk)[g*6 + 2*(ft%3) + hf, d, k*128 + p]
        wqk_cols = np.concatenate(
            [Wq[hs].reshape(HPC * D, C).T, Wk[hs].reshape(HPC * D, C).T],
            axis=1)                                        # [C, 768]
        wqk_sb = (wqk_cols.reshape(KT, 128, 2 * HPC * D)
                  .transpose(1, 0, 2).reshape(128, KT * C))
        # wv_sb[p, k*384 + h*64 + d] = Wv[g*6+h, d, k*128+p]
        wv_cols = Wv[hs].reshape(HPC * D, C).T             # [C, 384]
        wv_sb = (wv_cols.reshape(KT, 128, HPC * D)
                 .transpose(1, 0, 2).reshape(128, KT * HPC * D))
        # wp_sb[p, pq*768 + c] = W_proj[c, (g*6+2*pq)*64 + p]
        wproj = np.empty((HPC // 2, 128, C), np.float32)
        for p in range(HPC // 2):
            gh = g * HPC + 2 * p
            wproj[p, 0:64] = W_proj[:, gh * D:(gh + 1) * D].T
            wproj[p, 64:128] = W_proj[:, (gh + 1) * D:(gh + 2) * D].T
        wp_sb = wproj.reshape((HPC // 2) * 128, C).reshape(
            HPC // 2, 128, C).transpose(1, 0, 2).reshape(128, (HPC // 2) * C)
        # bq_sb[hf*64 + d, ft] = (bq | bk)[g*6 + 2*(ft%3) + hf, d]
        bq_sb = np.empty((128, HPC), np.float32)
        for ft in range(HPC):
            src = bq if ft < 3 else bk
            h0 = g * HPC + 2 * (ft % 3)
            bq_sb[0:64, ft] = src[h0]
            bq_sb[64:128, ft] = src[h0 + 1]
        per_group.append((
            np.ascontiguousarray(_f32_to_bf16(wqk_sb)),
            np.ascontiguousarray(_f32_to_bf16(wv_sb)),
            np.ascontiguousarray(_f32_to_bf16(wp_sb)),
            np.ascontiguousarray(bq_sb),
        ))
    # host-side constant: y += b_proj + W_proj @ b_v  (softmax rows sum to 1)
    y_const = b_proj + W_proj @ bvv                       # [C] f32
    prep = (per_group, y_const)
    _wprep_cache.clear()
    _wprep_cache[key] = (W_qkv, b_qkv, W_proj, b_proj, prep)
    return prep


def make_in_maps(x1, x2, W_qkv, b_qkv, W_proj, b_proj):
    """Host-side shard prep. Core c -> (batch c//2, head-group c%2)."""
    per_group, _ = _prep_weights(W_qkv, b_qkv, W_proj, b_proj)
    x1 = np.asarray(x1, np.float32)
    x2 = np.asarray(x2, np.float32)
    # x_sb[p, q*3072 + k*512 + t] = xT[k*128+p, q*512+t], bf16
    xs = []
    for x in (x1, x2):
        per_b = []
        for b in range(B):
            xt = _f32_to_bf16(np.ascontiguousarray(x[b].T))   # [768, 1024]
            per_b.append(np.ascontiguousarray(
                xt.reshape(KT, 128, NQ, 512).transpose(1, 2, 0, 3)
                .reshape(128, KT * N)))
            # note axis order (p, q, k, t): col = q*(KT*512) + k*512 + t
        xs.append(per_b)

    in_maps = []
    for c in range(8):
        b, g = divmod(c, 2)
        wqk_sb, wv_sb, wp_sb, bq_sb = per_group[g]
        in_maps.append({
            "wqk": wqk_sb,
            "wv": wv_sb,
            "wp": wp_sb,
            "x0": xs[0][b],
            "x1": xs[1][b],
            "bq": bq_sb,
        })
    return in_maps


def combine_outputs(results, y_const):
    y1 = np.empty((B, N, C), np.float32)
    y2 = np.empty((B, N, C), np.float32)
    for b in range(B):
        p0 = _bf16_to_f32(results[2 * b]["yp"])
        p1 = _bf16_to_f32(results[2 * b + 1]["yp"])
        ysum = p0 + p1                                     # [2C, N]
        y1[b] = ysum[0:C].T + y_const
        y2[b] = ysum[C:2 * C].T + y_const
    return y1, y2


def kernel(x1, x2, W_qkv, b_qkv, W_proj, b_proj):
    in_maps = make_in_maps(x1, x2, W_qkv, b_qkv, W_proj, b_proj)
    _, y_const = _prep_weights(W_qkv, b_qkv, W_proj, b_proj)
    try:
        results = _get_runner().run(in_maps)
    except Exception:
        # robust fallback: the one-shot path run_bass_kernel_spmd uses
        nc = _get_program()
        results = run_bass_kernel_spmd(
            nc, in_maps, core_ids=list(range(8))).results
    return combine_outputs(results, y_const)


# revision 41
# speedup vs baseline: 1.0609x; 1.0609x over previous
"""Dual-stream attention (nn_Attention2) on 8 TRN2 NeuronCores, v3.

Problem: B=4, N=1024, C=768, H=12, D=64.
  qkv_s = x_s @ W_qkv.T + b_qkv          (s = 1,2; shared weights)
  attn  = softmax(q1k1/sqrt(D) + q2k2/sqrt(D))   (one shared softmax)
  o_s   = attn @ v_s;  y_s = o_s @ W_proj.T + b_proj

Sharding: 8 cores = 4 batches x 2 head-groups (6 heads each). v3 drops
ALL on-device collectives: every core receives its full inputs (x for
its batch, weights for its head group) pre-staged in device DRAM in
SBUF-ready [128, free] layout, so SBUF loads are single contiguous DMAs
and the PE starts ~4us in (v2 burned ~30us on staging copies +
AllGathers). Each core emits its PARTIAL projection output (both
streams, bf16); the host adds the pair partials (the "unshard" step),
which replaces v2's tail ReduceScatter.

Bias handling: q/k biases are applied for free on the mandatory
psum->SBUF drains (ACT Identity-with-bias / DVE tensor_scalar).  v and
proj biases fold into a single host-side constant vector: softmax rows
sum to 1, so  o = attn@(v + bv) = attn@v + bv  and the bias lands in
y as  W_proj @ bv + b_proj, added on the host.

Compute phases (per core, stream-stacked [s1 64 | s2 64] q/k tiles so
combined scores are one matmul chain; attention in sT[k,q] orientation;
ones-matmul gives the softmax denominator broadcast across partitions):
  1. qkv: 12 q/k passes ([128,1024] psum, drains alternate DVE/ACT),
     16 v passes ([128,384] psum, plain-copy drains).
  2. attention per head: scores 2 tiles ahead, exp on ACT (bf16 out),
     AV accumulated into a single [128,1024] psum, bf16 pairwise
     add-tree on DVE + ones-matmul denominator in its OWN psum tag
     (v2 aliased it with the scores ring, serializing ~2us/head),
     reciprocal+normalize on DVE, finalize deferred one head.
  3. projection: per (q,cb,s) 3-matmul chains, Copy drains on ACT/DVE,
     partial y DMAed straight out (no collective, no device bias).
bf16 matmuls, f32 PSUM.
"""

import contextlib
import threading

import numpy as np
import ml_dtypes
import jax
from jax.sharding import Mesh, PartitionSpec
try:
    from jax.experimental.shard_map import shard_map
except ImportError:
    from jax.sharding import shard_map

import concourse.bass as bass
import concourse.tile as tile
from concourse import bacc, mybir
from concourse.bass_utils import run_bass_kernel_spmd
from concourse.bass2jax import (
    install_neuronx_cc_hook,
    partition_id_tensor,
    _bass_exec_p,
)

F32 = mybir.dt.float32
BF16 = mybir.dt.bfloat16
AL = mybir.AluOpType
AF = mybir.ActivationFunctionType

B, N, C, H = 4, 1024, 768, 12
D = C // H              # 64
HPC = 6                 # heads per core
KT = C // 128           # 6 contraction tiles over C
NQ = N // 512           # 2 q-halves
NK = N // 128           # 8 k-blocks
SCALE = float(D) ** -0.5
BF = ml_dtypes.bfloat16
WARM_QK = 36    # PE p-state warmup matmuls before the qk block
WARM_PROJ = 14  # warmup matmuls bridging the last finalize -> proj


def build_program(loop_reps=0, phase_cut=None, use_cc=True):
    """use_cc is accepted for test.py compat; v3 has no collectives, so
    the timing build and the graded build are the same program."""
    del use_cc
    nc = bacc.Bacc("TRN2", target_bir_lowering=False, debug=False)

    # inputs pre-staged in DRAM in SBUF layout ([128, free], bf16)
    wqk_d = nc.dram_tensor("wqk", [128, KT * C], BF16, kind="ExternalInput").ap()
    wv_d = nc.dram_tensor("wv", [128, KT * HPC * D], BF16,
                          kind="ExternalInput").ap()
    wp_d = nc.dram_tensor("wp", [128, (HPC // 2) * C], BF16,
                          kind="ExternalInput").ap()
    x_d = [nc.dram_tensor(f"x{s}", [128, KT * N], BF16,
                          kind="ExternalInput").ap() for s in range(2)]
    bq_d = nc.dram_tensor("bq", [128, HPC], F32, kind="ExternalInput").ap()
    yp = nc.dram_tensor("yp", [2 * C, N], BF16, kind="ExternalOutput").ap()

    with tile.TileContext(nc) as tc:
        with (
            tc.tile_pool(name="persist", bufs=1) as pp,
            tc.tile_pool(name="expp", bufs=16) as ep,
            tc.tile_pool(name="rdp", bufs=2) as rp,
            tc.tile_pool(name="ybp", bufs=4) as yp_pool,
            tc.For_i(0, loop_reps, 1) if loop_reps else contextlib.nullcontext(),
        ):
            # ---- SBUF loads: contiguous DMAs straight from DRAM inputs ----
            wqk_sb = pp.tile([128, KT * C], BF16, tag="wqk", name="wqk")
            x_sb = [pp.tile([128, KT * N], BF16, tag=f"x{s}", name=f"x{s}")
                    for s in range(2)]
            wv_sb = pp.tile([128, KT * HPC * D], BF16, tag="wv", name="wv")
            wp_sb = pp.tile([128, (HPC // 2) * C], BF16, tag="wp", name="wp")
            bq_sb = pp.tile([128, HPC], F32, tag="bq")
            # The DMA fabric is effectively one serial ~360GB/s resource and
            # cross-queue arbitration scrambles ordering, so issue ALL input
            # loads on one queue in exactly the order compute consumes them.
            half = KT * 512
            for k in range(KT):
                nc.sync.dma_start(out=wqk_sb[:, k * C:(k + 1) * C],
                                  in_=wqk_d[:, k * C:(k + 1) * C])
            nc.sync.dma_start(out=x_sb[0][:, 0:half], in_=x_d[0][:, 0:half])
            nc.sync.dma_start(out=bq_sb, in_=bq_d)
            nc.sync.dma_start(out=x_sb[0][:, half:2 * half],
                              in_=x_d[0][:, half:2 * half])
            nc.sync.dma_start(out=x_sb[1][:, 0:half], in_=x_d[1][:, 0:half])
            nc.sync.dma_start(out=x_sb[1][:, half:2 * half],
                              in_=x_d[1][:, half:2 * half])
            nc.sync.dma_start(out=wv_sb, in_=wv_d)
            nc.sync.dma_start(out=wp_sb, in_=wp_d)
            ones = pp.tile([128, 128], BF16, tag="ones")
            nc.vector.memset(ones, 1.0)
            # PE p-state warmup: the tensor engine needs ~3us of continuous
            # execution to ramp 0.65 -> 2.4 GHz, and any idle resets it.
            # Dummy matmuls from t~0.5us keep it pinned hot until the first
            # x chunk lands (~9us), so real passes start at full clock.
            warm = pp.tile([128, 512], BF16, tag="warm")
            nc.vector.memset(warm, 0.0)

            # x_sb column layout: q*(KT*512) + k*512 + t  (t in 0..511)
            def xcol(s, q, k, t0, n):
                base = q * (KT * 512) + k * 512 + t0
                return x_sb[s][:, base:base + n]

            # ---- phase 1: q/k then v projections ----
            qt = [pp.tile([128, N], BF16, tag=f"qt{h}", name=f"qt{h}")
                  for h in range(HPC)]
            kt_ = [pp.tile([128, N], BF16, tag=f"kt{h}", name=f"kt{h}")
                   for h in range(HPC)]
            vt = [pp.tile([128, HPC * 128], BF16, tag=f"vt{t}", name=f"vt{t}")
                  for t in range(NK)]

            # ps_s is allocated FIRST so its banks never alias the qkv
            # pools' -- head 0/1 score tiles must not wait on qk drains.
            ps_s = tc.alloc_tile_pool(name="ps_s", bufs=2, space="PSUM")
            ps_qk = tc.alloc_tile_pool(name="ps_qk", bufs=4, space="PSUM")

            def qk_pass(ft, s, q):
                # one q-half per pass: [128,512] psum (1 bank), 6 matmuls
                p = ps_qk.tile([128, 512], F32, tag="qkp", name="qkp")
                for k in range(KT):
                    nc.tensor.matmul(
                        p,
                        lhsT=wqk_sb[:, k * C + ft * 128:
                                    k * C + (ft + 1) * 128],
                        rhs=xcol(s, q, k, 0, 512),
                        start=(k == 0), stop=(k == KT - 1))
                pair = qt if ft < HPC // 2 else kt_
                h0 = (ft % (HPC // 2)) * 2
                ql = slice(q * 512, (q + 1) * 512)
                # one drain per head-half: hf=0 on DVE, hf=1 on ACT
                nc.vector.tensor_scalar(
                    out=pair[h0][s * 64:(s + 1) * 64, ql],
                    in0=p[0:64, :],
                    scalar1=bq_sb[0:64, ft:ft + 1],
                    scalar2=None, op0=AL.add)
                nc.scalar.activation(
                    out=pair[h0 + 1][s * 64:(s + 1) * 64, ql],
                    in_=p[64:128, :],
                    func=AF.Identity,
                    bias=bq_sb[64:128, ft:ft + 1])

            def v_pass(s, t):
                p = ps_v.tile([128, HPC * D], F32, tag="vp", name="vp")
                q = t // 4
                t0 = t * 128 - q * 512
                for k in range(KT):
                    nc.tensor.matmul(
                        p,
                        lhsT=xcol(s, q, k, t0, 128),
                        rhs=wv_sb[:, k * HPC * D:(k + 1) * HPC * D],
                        start=(k == 0), stop=(k == KT - 1))
                out3 = vt[t].rearrange(
                    "p (h two d) -> p h two d", two=2, d=D)[:, :, s, :]
                src = p.rearrange("p (h d) -> p h d", d=D)
                # all v drains on DVE: ACT is running head-0/1 exps by now
                nc.vector.tensor_copy(out=out3, in_=src)

            # (s, q) outer so stream 0's passes only need x0 (arrives
            # first); ft ordered so head 0/1's q AND k finish earliest --
            # the scheduler then starts head-0 scores/exp during the
            # qkv tail. Live psum banks: qk phase sp(4)+qk(4)=8; v phase
            # sp(4)+op2(2)+v(2)=8; ps_d reuses v's banks after release.
            for i in range(WARM_QK):
                pw = ps_qk.tile([128, 512], F32, tag="qkp", name="warm")
                nc.tensor.matmul(pw, lhsT=ones, rhs=warm,
                                 start=True, stop=True)
            for s in range(2):
                for q in range(NQ):
                    for ft in (0, 3, 1, 4, 2, 5):
                        qk_pass(ft, s, q)
            ps_qk.release()
            dsp = tc.alloc_tile_pool(name="dsp", bufs=2)
            ps_o = tc.alloc_tile_pool(name="ps_o", bufs=1, space="PSUM")
            ps_v = tc.alloc_tile_pool(name="ps_v", bufs=2, space="PSUM")
            # v passes for t>=2 are woven into head 0's loop below (PE
            # filler that keeps the tensor engine warm while the exp
            # stream paces); the debug cut emits them all here instead.
            if phase_cut == "qkv":
                for t in range(NK):
                    for s in range(2):
                        v_pass(s, t)
                ps_v.release()
                for h in range(HPC):
                    nc.sync.dma_start(out=yp[h * 128:(h + 1) * 128, 0:512],
                                      in_=qt[h][:, 0:512])
                    nc.sync.dma_start(out=yp[h * 128:(h + 1) * 128, 512:1024],
                                      in_=kt_[h][:, 0:512])
                for t in range(NK):
                    nc.sync.dma_start(
                        out=yp[C + (t % 6) * 128:C + (t % 6 + 1) * 128,
                               (t // 6) * 128:(t // 6) * 128 + 128],
                        in_=vt[t][:, 0:128])

            if phase_cut in (None, "attn"):
                # ---- phase 2: attention per head, sT[k, q] orientation ----
                # The exp stream on ACT is the pacer (8 x ~1.04us per head);
                # everything else is scheduled to never starve it:
                #  * scores for kb+2 are emitted ahead of all other PE work
                #    in each iteration (2-deep psum prefetch ring),
                #  * AV accumulation is rotated to start at kb5 and the
                #    kb0..4 products are deferred into the NEXT head's first
                #    two iterations, so the op2 psum buffer (single, PSUM is
                #    full) is reused only after the previous head's
                #    normalize has cleared -- no boundary stall,
                #  * the denominator uses a running sum (one DVE add after
                #    the last exp instead of a 3-level tree tail) and the
                #    finalize chain (ones-matmul -> reciprocal -> normalize,
                #    q-half split) is emitted at kb1 of the next head.
                ost = [[pp.tile([128, N], BF16, tag=f"ost{s}_{p}",
                                name=f"ost{s}_{p}")
                        for p in range(HPC // 2)] for s in range(2)]
                ps_d = None  # allocated at the h==1 boundary (reuses ps_v)

                AVS = 5  # first kb of the in-head AV accumulation window

                def scores_tile(h, kb, use_dp=False):
                    # use_dp: time-share the denominator psum banks for the
                    # next head's kb0 tile -- a 3rd effective prefetch slot
                    # that removes the head-boundary WAR stall on sp0.
                    if use_dp:
                        spt = ps_d.tile([128, N], F32, tag="dp",
                                        name="sp", bufs=1)
                    else:
                        spt = ps_s.tile([128, N], F32, tag=f"sp{kb % 2}",
                                        name="sp", bufs=1)
                    for q in range(NQ):
                        nc.tensor.matmul(
                            spt[:, q * 512:(q + 1) * 512],
                            lhsT=kt_[h][:, kb * 128:(kb + 1) * 128],
                            rhs=qt[h][:, q * 512:(q + 1) * 512],
                            start=True, stop=True)
                    return spt

                def av(h, kb, ex_t, op2_t):
                    for q in range(NQ):
                        nc.tensor.matmul(
                            op2_t[:, q * 512:(q + 1) * 512],
                            lhsT=vt[kb][:, h * 128:(h + 1) * 128],
                            rhs=ex_t[:, q * 512:(q + 1) * 512],
                            start=(kb == AVS), stop=(kb == AVS - 1))

                def fin_emit(h, st):
                    dp2 = ps_d.tile([128, N], F32, tag="dp",
                                    name="dp2", bufs=1)
                    rd = rp.tile([128, N], F32, tag="rd", name="rd")
                    for q in range(NQ):
                        ql = slice(q * 512, (q + 1) * 512)
                        nc.tensor.matmul(
                            dp2[:, ql], lhsT=ones, rhs=st["es"][:, ql],
                            start=True, stop=True)
                        nc.vector.reciprocal_approx_fast(
                            out=rd[:, ql], in_=dp2[:, ql])
                        for s in range(2):
                            nc.vector.tensor_mul(
                                out=ost[s][h // 2][(h % 2) * 64:
                                                   (h % 2) * 64 + 64, ql],
                                in0=st["op2"][s * 64:(s + 1) * 64, ql],
                                in1=rd[s * 64:(s + 1) * 64, ql])

                prev = None
                sp01 = [scores_tile(0, 0), scores_tile(0, 1)]
                for t in (0, 1):
                    for s in range(2):
                        v_pass(s, t)
                for h in range(HPC):
                    if h == 1:
                        # all v psum work was emitted in head 0's loop
                        ps_v.release()
                        ps_d = tc.alloc_tile_pool(name="ps_d", bufs=1,
                                                  space="PSUM")
                    op2 = ps_o.tile([128, N], F32, tag="op2", name="op2")
                    sp = [None] * NK
                    sp[0], sp[1] = sp01
                    ex = [None] * NK
                    run = None
                    es = None
                    for kb in range(NK):
                        ex[kb] = ep.tile([128, N], BF16, tag="exp", name="exp")
                        nc.scalar.activation(out=ex[kb], in_=sp[kb],
                                             func=AF.Exp)
                        if kb + 2 < NK:
                            sp[kb + 2] = scores_tile(h, kb + 2)
                        if h == 0 and kb + 2 < NK:
                            # weave the remaining v passes into head 0
                            v_pass(0, kb + 2)
                            v_pass(1, kb + 2)
                        if prev is not None:
                            if kb == 0:
                                for dkb in range(3):
                                    av(h - 1, dkb, prev["ex"][dkb],
                                       prev["op2"])
                            elif kb == 1:
                                for dkb in range(3, AVS):
                                    av(h - 1, dkb, prev["ex"][dkb],
                                       prev["op2"])
                                fin_emit(h - 1, prev)
                        if h + 1 < HPC:
                            # pre-emit next head's first score tiles ahead
                            # of this head's last AVs so exp(h+1,0) starts
                            # with zero bubble at the boundary; from h>=1
                            # kb0 goes through the dp banks (free until the
                            # next fin), which unpins the sp0 WAR entirely
                            if h >= 1 and kb == AVS:
                                sp01 = [scores_tile(h + 1, 0, use_dp=True),
                                        None]
                            elif h == 0 and kb == NK - 2:
                                sp01 = [scores_tile(h + 1, 0), None]
                            elif kb == NK - 1:
                                sp01[1] = scores_tile(h + 1, 1)
                        if kb >= AVS:
                            av(h, kb, ex[kb], op2)
                            if h == HPC - 1:
                                # last head: fold the deferred AVs into the
                                # in-head window so finalize fires at once
                                for dkb in range(2 * (kb - AVS),
                                                 min(2 * (kb - AVS) + 2, AVS)):
                                    av(h, dkb, ex[dkb], op2)
                        # running-sum denominator on DVE
                        if kb == 1:
                            run = dsp.tile([128, N], BF16, tag="run",
                                           name="run")
                            nc.vector.tensor_tensor(out=run, in0=ex[0],
                                                    in1=ex[1], op=AL.add)
                        elif 2 <= kb <= NK - 2:
                            nrun = dsp.tile([128, N], BF16, tag="run",
                                            name="run")
                            nc.vector.tensor_tensor(out=nrun, in0=run,
                                                    in1=ex[kb], op=AL.add)
                            run = nrun
                        else:
                            if kb == NK - 1:
                                es = dsp.tile([128, N], BF16, tag="es",
                                              name="es")
                                nc.vector.tensor_tensor(out=es, in0=run,
                                                        in1=ex[kb], op=AL.add)
                    prev = {"op2": op2, "ex": ex, "es": es}
                # last head's AVs were folded in-loop; just finalize
                fin_emit(HPC - 1, prev)
                ps_d.release()
                ps_o.release()
                dsp.release()
                ps_s.release()

                if phase_cut == "attn":
                    for s in range(2):
                        for p_ in range(HPC // 2):
                            nc.sync.dma_start(
                                out=yp[(s * 3 + p_) * 128:
                                       (s * 3 + p_ + 1) * 128, :],
                                in_=ost[s][p_])

            if phase_cut is None:
                # ---- phase 3: projection; partial y straight to DRAM ----
                # (cb,s) outer / q inner so both q-halves land in one yb
                # tile -> 12 full-row DMAs, split across two queues so
                # descriptor generation doesn't pace the tail.
                ps_y = tc.alloc_tile_pool(name="ps_y", bufs=6, space="PSUM")
                NP = HPC // 2
                # bridge the last finalize chain (~3.5us of serial DVE work)
                # with dummy matmuls so the PE clock stays hot into the proj
                for i in range(WARM_PROJ):
                    pw = ps_y.tile([128, 512], F32, tag="yp", name="warm")
                    nc.tensor.matmul(pw, lhsT=ones, rhs=warm,
                                     start=True, stop=True)
                for cb in range(C // 128):
                    for s in range(2):
                        yb = yp_pool.tile([128, N], BF16, tag="yb")
                        for q in range(NQ):
                            py = ps_y.tile([128, 512], F32, tag="yp",
                                           name="yp")
                            for p in range(NP):
                                nc.tensor.matmul(
                                    py,
                                    lhsT=wp_sb[:, p * C + cb * 128:
                                               p * C + (cb + 1) * 128],
                                    rhs=ost[s][p][:, q * 512:(q + 1) * 512],
                                    start=(p == 0), stop=(p == NP - 1))
                            if (q + s) % 2 == 0:
                                nc.scalar.activation(
                                    out=yb[:, q * 512:(q + 1) * 512],
                                    in_=py, func=AF.Copy)
                            else:
                                nc.vector.tensor_copy(
                                    out=yb[:, q * 512:(q + 1) * 512], in_=py)
                        eng = nc.sync if (cb + s) % 2 == 0 else nc.gpsimd
                        eng.dma_start(
                            out=yp[s * C + cb * 128:s * C + (cb + 1) * 128, :],
                            in_=yb)
                ps_y.release()

    nc.compile()
    return nc


_cache = threading.Lock()
_nc = None
_runner = None


def _get_program():
    global _nc
    with _cache:
        if _nc is None:
            _nc = build_program()
    return _nc


class _Runner:
    """Compile the 8-core sharded PJRT callable once and reuse it across
    kernel() calls (run_bass_kernel_spmd re-traces jax.jit per call, which
    costs seconds; the NEFF itself is what actually runs)."""

    def __init__(self, nc, n_cores=8):
        install_neuronx_cc_hook()
        self.nc = nc
        self.n_cores = n_cores
        partition_name = (nc.partition_id_tensor.name
                          if nc.partition_id_tensor else None)
        in_names, out_names, out_avals, zero_outs = [], [], [], []
        for alloc in nc.m.functions[0].allocations:
            if not isinstance(alloc, mybir.MemoryLocationSet):
                continue
            name = alloc.memorylocations[0].name
            if alloc.kind == "ExternalInput":
                if name != partition_name:
                    in_names.append(name)
            elif alloc.kind == "ExternalOutput":
                out_names.append(name)
                shape = tuple(alloc.tensor_shape)
                dtype = mybir.dt.np(alloc.dtype)
                out_avals.append(jax.core.ShapedArray(shape, dtype))
                zero_outs.append(
                    np.zeros((n_cores * shape[0], *shape[1:]), dtype))
        self.in_names = in_names
        self.out_names = out_names
        self.out_shapes = [tuple(a.shape) for a in out_avals]
        self.zero_outs = zero_outs
        n_params = len(in_names)
        n_outs = len(out_avals)
        all_in = list(in_names) + list(out_names)
        if partition_name is not None:
            all_in.append(partition_name)

        def _body(*args):
            operands = list(args)
            if partition_name is not None:
                operands.append(partition_id_tensor())
            outs = _bass_exec_p.bind(
                *operands,
                out_avals=tuple(out_avals),
                in_names=tuple(all_in),
                out_names=tuple(out_names),
                lowering_input_output_aliases=(),
                sim_require_finite=True,
                sim_require_nnan=True,
                nc=nc,
            )
            return tuple(outs)

        devices = jax.devices()[:n_cores]
        mesh = Mesh(np.asarray(devices), ("core",))
        self.f = jax.jit(
            shard_map(
                _body, mesh=mesh,
                in_specs=(PartitionSpec("core"),) * (n_params + n_outs),
                out_specs=(PartitionSpec("core"),) * n_outs,
                check_rep=False,
            ),
            keep_unused=True,
        )

    def run(self, in_maps):
        n = self.n_cores
        concat_in = [
            np.concatenate([np.asarray(in_maps[c][name]) for c in range(n)],
                           axis=0)
            for name in self.in_names
        ]
        out_arrs = self.f(*concat_in, *self.zero_outs)
        return [
            {name: np.asarray(out_arrs[i]).reshape(n, *self.out_shapes[i])[c]
             for i, name in enumerate(self.out_names)}
            for c in range(n)
        ]


def _get_runner():
    global _runner
    nc = _get_program()
    with _cache:
        if _runner is None:
            _runner = _Runner(nc)
    return _runner


def _f32_to_bf16(a):
    """Fast round-to-nearest f32->bf16 via integer ops (contiguous input)."""
    u = np.ascontiguousarray(a, np.float32).view(np.uint32)
    return (((u + 0x7FFF) + ((u >> 16) & 1)) >> 16).astype(np.uint16).view(BF)


def _bf16_to_f32(a):
    return (np.asarray(a).view(np.uint16).astype(np.uint32) << 16).view(
        np.float32)


_wprep_cache = {}


def _prep_weights(W_qkv, b_qkv, W_proj, b_proj):
    key = (id(W_qkv), id(b_qkv), id(W_proj), id(b_proj))
    hit = _wprep_cache.get(key)
    if hit is not None and (hit[0] is W_qkv and hit[1] is b_qkv
                            and hit[2] is W_proj and hit[3] is b_proj):
        return hit[4]
    W_qkv = np.asarray(W_qkv, np.float32)
    b_qkv = np.asarray(b_qkv, np.float32)
    W_proj = np.asarray(W_proj, np.float32)
    b_proj = np.asarray(b_proj, np.float32)
    Wq = W_qkv[0:C].reshape(H, D, C) * SCALE
    Wk = W_qkv[C:2 * C].reshape(H, D, C)
    Wv = W_qkv[2 * C:3 * C].reshape(H, D, C)
    bq = b_qkv[0:C].reshape(H, D) * SCALE
    bk = b_qkv[C:2 * C].reshape(H, D)
    bvv = b_qkv[2 * C:3 * C]

    per_group = []
    for g in range(2):
        hs = slice(g * HPC, (g + 1) * HPC)
        # wqk_sb[p, k*768 + ft*128 + hf*64 + d] =
        #   (Wq | Wk)[g*6 + 2*(ft%3) + hf, d, k*128 + p]
        wqk_cols = np.concatenate(
            [Wq[hs].reshape(HPC * D, C).T, Wk[hs].reshape(HPC * D, C).T],
            axis=1)                                        # [C, 768]
        wqk_sb = (wqk_cols.reshape(KT, 128, 2 * HPC * D)
                  .transpose(1, 0, 2).reshape(128, KT * C))
        # wv_sb[p, k*384 + h*64 + d] = Wv[g*6+h, d, k*128+p]
        wv_cols = Wv[hs].reshape(HPC * D, C).T             # [C, 384]
        wv_sb = (wv_cols.reshape(KT, 128, HPC * D)
                 .transpose(1, 0, 2).reshape(128, KT * HPC * D))
        # wp_sb[p, pq*768 + c] = W_proj[c, (g*6+2*pq)*64 + p]
        wproj = np.empty((HPC // 2, 128, C), np.float32)
        for p in range(HPC // 2):
            gh = g * HPC + 2 * p
            wproj[p, 0:64] = W_proj[:, gh * D:(gh + 1) * D].T
            wproj[p, 64:128] = W_proj[:, (gh + 1) * D:(gh + 2) * D].T
        wp_sb = wproj.reshape((HPC // 2) * 128, C).reshape(
            HPC // 2, 128, C).transpose(1, 0, 2).reshape(128, (HPC // 2) * C)
        # bq_sb[hf*64 + d, ft] = (bq | bk)[g*6 + 2*(ft%3) + hf, d]
        bq_sb = np.empty((128, HPC), np.float32)
        for ft in range(HPC):
            src = bq if ft < 3 else bk
            h0 = g * HPC + 2 * (ft % 3)
            bq_sb[0:64, ft] = src[h0]
            bq_sb[64:128, ft] = src[h0 + 1]
        per_group.append((
            np.ascontiguousarray(_f32_to_bf16(wqk_sb)),
            np.ascontiguousarray(_f32_to_bf16(wv_sb)),
            np.ascontiguousarray(_f32_to_bf16(wp_sb)),
            np.ascontiguousarray(bq_sb),
        ))
    # host-side constant: y += b_proj + W_proj @ b_v  (softmax rows sum to 1)
    y_const = b_proj + W_proj @ bvv                       # [C] f32
    prep = (per_group, y_const)
    _wprep_cache.clear()
    _wprep_cache[key] = (W_qkv, b_qkv, W_proj, b_proj, prep)
    return prep


def make_in_maps(x1, x2, W_qkv, b_qkv, W_proj, b_proj):
    """Host-side shard prep. Core c -> (batch c//2, head-group c%2)."""
    per_group, _ = _prep_weights(W_qkv, b_qkv, W_proj, b_proj)
    x1 = np.asarray(x1, np.float32)
    x2 = np.asarray(x2, np.float32)
    # x_sb[p, q*3072 + k*512 + t] = xT[k*128+p, q*512+t], bf16
    xs = []
    for x in (x1, x2):
        per_b = []
        for b in range(B):
            xt = _f32_to_bf16(np.ascontiguousarray(x[b].T))   # [768, 1024]
            per_b.append(np.ascontiguousarray(
                xt.reshape(KT, 128, NQ, 512).transpose(1, 2, 0, 3)
                .reshape(128, KT * N)))
            # note axis order (p, q, k, t): col = q*(KT*512) + k*512 + t
        xs.append(per_b)

    in_maps = []
    for c in range(8):
        b, g = divmod(c, 2)
        wqk_sb, wv_sb, wp_sb, bq_sb = per_group[g]
        in_maps.append({
            "wqk": wqk_sb,
            "wv": wv_sb,
            "wp": wp_sb,
            "x0": xs[0][b],
            "x1": xs[1][b],
            "bq": bq_sb,
        })
    return in_maps


def combine_outputs(results, y_const):
    y1 = np.empty((B, N, C), np.float32)
    y2 = np.empty((B, N, C), np.float32)
    for b in range(B):
        p0 = _bf16_to_f32(results[2 * b]["yp"])
        p1 = _bf16_to_f32(results[2 * b + 1]["yp"])
        ysum = p0 + p1                                     # [2C, N]
        y1[b] = ysum[0:C].T + y_const
        y2[b] = ysum[C:2 * C].T + y_const
    return y1, y2


def kernel(x1, x2, W_qkv, b_qkv, W_proj, b_proj):
    in_maps = make_in_maps(x1, x2, W_qkv, b_qkv, W_proj, b_proj)
    _, y_const = _prep_weights(W_qkv, b_qkv, W_proj, b_proj)
    try:
        results = _get_runner().run(in_maps)
    except Exception:
        # robust fallback: the one-shot path run_bass_kernel_spmd uses
        nc = _get_program()
        results = run_bass_kernel_spmd(
            nc, in_maps, core_ids=list(range(8))).results
    return combine_outputs(results, y_const)


# revision 42
# speedup vs baseline: 1.0724x; 1.0109x over previous
"""Dual-stream attention (nn_Attention2) on 8 TRN2 NeuronCores, v3.

Problem: B=4, N=1024, C=768, H=12, D=64.
  qkv_s = x_s @ W_qkv.T + b_qkv          (s = 1,2; shared weights)
  attn  = softmax(q1k1/sqrt(D) + q2k2/sqrt(D))   (one shared softmax)
  o_s   = attn @ v_s;  y_s = o_s @ W_proj.T + b_proj

Sharding: 8 cores = 4 batches x 2 head-groups (6 heads each). v3 drops
ALL on-device collectives: every core receives its full inputs (x for
its batch, weights for its head group) pre-staged in device DRAM in
SBUF-ready [128, free] layout, so SBUF loads are single contiguous DMAs
and the PE starts ~4us in (v2 burned ~30us on staging copies +
AllGathers). Each core emits its PARTIAL projection output (both
streams, bf16); the host adds the pair partials (the "unshard" step),
which replaces v2's tail ReduceScatter.

Bias handling: q/k biases are applied for free on the mandatory
psum->SBUF drains (ACT Identity-with-bias / DVE tensor_scalar).  v and
proj biases fold into a single host-side constant vector: softmax rows
sum to 1, so  o = attn@(v + bv) = attn@v + bv  and the bias lands in
y as  W_proj @ bv + b_proj, added on the host.

Compute phases (per core, stream-stacked [s1 64 | s2 64] q/k tiles so
combined scores are one matmul chain; attention in sT[k,q] orientation;
ones-matmul gives the softmax denominator broadcast across partitions):
  1. qkv: 12 q/k passes ([128,1024] psum, drains alternate DVE/ACT),
     16 v passes ([128,384] psum, plain-copy drains).
  2. attention per head: scores 2 tiles ahead, exp on ACT (bf16 out),
     AV accumulated into a single [128,1024] psum, bf16 pairwise
     add-tree on DVE + ones-matmul denominator in its OWN psum tag
     (v2 aliased it with the scores ring, serializing ~2us/head),
     reciprocal+normalize on DVE, finalize deferred one head.
  3. projection: per (q,cb,s) 3-matmul chains, Copy drains on ACT/DVE,
     partial y DMAed straight out (no collective, no device bias).
bf16 matmuls, f32 PSUM.
"""

import contextlib
import threading

import numpy as np
import ml_dtypes
import jax
from jax.sharding import Mesh, PartitionSpec
try:
    from jax.experimental.shard_map import shard_map
except ImportError:
    from jax.sharding import shard_map

import concourse.bass as bass
import concourse.tile as tile
from concourse import bacc, mybir
from concourse.bass_utils import run_bass_kernel_spmd
from concourse.bass2jax import (
    install_neuronx_cc_hook,
    partition_id_tensor,
    _bass_exec_p,
)

F32 = mybir.dt.float32
BF16 = mybir.dt.bfloat16
AL = mybir.AluOpType
AF = mybir.ActivationFunctionType

B, N, C, H = 4, 1024, 768, 12
D = C // H              # 64
HPC = 6                 # heads per core
KT = C // 128           # 6 contraction tiles over C
NQ = N // 512           # 2 q-halves
NK = N // 128           # 8 k-blocks
SCALE = float(D) ** -0.5
BF = ml_dtypes.bfloat16
WARM_QK = 0    # PE p-state warmup matmuls before the qk block
WARM_PROJ = 0  # warmup matmuls bridging the last finalize -> proj


def build_program(loop_reps=0, phase_cut=None, use_cc=True):
    """use_cc is accepted for test.py compat; v3 has no collectives, so
    the timing build and the graded build are the same program."""
    del use_cc
    nc = bacc.Bacc("TRN2", target_bir_lowering=False, debug=False)

    # inputs pre-staged in DRAM in SBUF layout ([128, free], bf16)
    wqk_d = nc.dram_tensor("wqk", [128, KT * C], BF16, kind="ExternalInput").ap()
    wv_d = nc.dram_tensor("wv", [128, KT * HPC * D], BF16,
                          kind="ExternalInput").ap()
    wp_d = nc.dram_tensor("wp", [128, (HPC // 2) * C], BF16,
                          kind="ExternalInput").ap()
    x_d = [nc.dram_tensor(f"x{s}", [128, KT * N], BF16,
                          kind="ExternalInput").ap() for s in range(2)]
    bq_d = nc.dram_tensor("bq", [128, HPC], F32, kind="ExternalInput").ap()
    yp = nc.dram_tensor("yp", [2 * C, N], BF16, kind="ExternalOutput").ap()

    with tile.TileContext(nc) as tc:
        with (
            tc.tile_pool(name="persist", bufs=1) as pp,
            tc.tile_pool(name="expp", bufs=16) as ep,
            tc.tile_pool(name="rdp", bufs=2) as rp,
            tc.tile_pool(name="ybp", bufs=4) as yp_pool,
            tc.For_i(0, loop_reps, 1) if loop_reps else contextlib.nullcontext(),
        ):
            # ---- SBUF loads: contiguous DMAs straight from DRAM inputs ----
            wqk_sb = pp.tile([128, KT * C], BF16, tag="wqk", name="wqk")
            x_sb = [pp.tile([128, KT * N], BF16, tag=f"x{s}", name=f"x{s}")
                    for s in range(2)]
            wv_sb = pp.tile([128, KT * HPC * D], BF16, tag="wv", name="wv")
            wp_sb = pp.tile([128, (HPC // 2) * C], BF16, tag="wp", name="wp")
            bq_sb = pp.tile([128, HPC], F32, tag="bq")
            # The DMA fabric is effectively one serial ~360GB/s resource and
            # cross-queue arbitration scrambles ordering, so issue ALL input
            # loads on one queue in exactly the order compute consumes them.
            half = KT * 512
            for k in range(KT):
                nc.sync.dma_start(out=wqk_sb[:, k * C:(k + 1) * C],
                                  in_=wqk_d[:, k * C:(k + 1) * C])
            nc.sync.dma_start(out=x_sb[0][:, 0:half], in_=x_d[0][:, 0:half])
            nc.sync.dma_start(out=bq_sb, in_=bq_d)
            nc.sync.dma_start(out=x_sb[0][:, half:2 * half],
                              in_=x_d[0][:, half:2 * half])
            nc.sync.dma_start(out=x_sb[1][:, 0:half], in_=x_d[1][:, 0:half])
            nc.sync.dma_start(out=x_sb[1][:, half:2 * half],
                              in_=x_d[1][:, half:2 * half])
            nc.sync.dma_start(out=wv_sb, in_=wv_d)
            nc.sync.dma_start(out=wp_sb, in_=wp_d)
            ones = pp.tile([128, 128], BF16, tag="ones")
            nc.vector.memset(ones, 1.0)
            # PE p-state warmup: the tensor engine needs ~3us of continuous
            # execution to ramp 0.65 -> 2.4 GHz, and any idle resets it.
            # Dummy matmuls from t~0.5us keep it pinned hot until the first
            # x chunk lands (~9us), so real passes start at full clock.
            warm = pp.tile([128, 512], BF16, tag="warm")
            nc.vector.memset(warm, 0.0)

            # x_sb column layout: q*(KT*512) + k*512 + t  (t in 0..511)
            def xcol(s, q, k, t0, n):
                base = q * (KT * 512) + k * 512 + t0
                return x_sb[s][:, base:base + n]

            # ---- phase 1: q/k then v projections ----
            qt = [pp.tile([128, N], BF16, tag=f"qt{h}", name=f"qt{h}")
                  for h in range(HPC)]
            kt_ = [pp.tile([128, N], BF16, tag=f"kt{h}", name=f"kt{h}")
                   for h in range(HPC)]
            vt = [pp.tile([128, HPC * 128], BF16, tag=f"vt{t}", name=f"vt{t}")
                  for t in range(NK)]

            # ps_s is allocated FIRST so its banks never alias the qkv
            # pools' -- head 0/1 score tiles must not wait on qk drains.
            ps_s = tc.alloc_tile_pool(name="ps_s", bufs=2, space="PSUM")
            ps_qk = tc.alloc_tile_pool(name="ps_qk", bufs=4, space="PSUM")

            def qk_pass(ft, s, q):
                # one q-half per pass: [128,512] psum (1 bank), 6 matmuls
                p = ps_qk.tile([128, 512], F32, tag="qkp", name="qkp")
                for k in range(KT):
                    nc.tensor.matmul(
                        p,
                        lhsT=wqk_sb[:, k * C + ft * 128:
                                    k * C + (ft + 1) * 128],
                        rhs=xcol(s, q, k, 0, 512),
                        start=(k == 0), stop=(k == KT - 1))
                pair = qt if ft < HPC // 2 else kt_
                h0 = (ft % (HPC // 2)) * 2
                ql = slice(q * 512, (q + 1) * 512)
                # one drain per head-half: hf=0 on DVE, hf=1 on ACT
                nc.vector.tensor_scalar(
                    out=pair[h0][s * 64:(s + 1) * 64, ql],
                    in0=p[0:64, :],
                    scalar1=bq_sb[0:64, ft:ft + 1],
                    scalar2=None, op0=AL.add)
                nc.scalar.activation(
                    out=pair[h0 + 1][s * 64:(s + 1) * 64, ql],
                    in_=p[64:128, :],
                    func=AF.Identity,
                    bias=bq_sb[64:128, ft:ft + 1])

            def v_pass(s, t):
                p = ps_v.tile([128, HPC * D], F32, tag="vp", name="vp")
                q = t // 4
                t0 = t * 128 - q * 512
                for k in range(KT):
                    nc.tensor.matmul(
                        p,
                        lhsT=xcol(s, q, k, t0, 128),
                        rhs=wv_sb[:, k * HPC * D:(k + 1) * HPC * D],
                        start=(k == 0), stop=(k == KT - 1))
                out3 = vt[t].rearrange(
                    "p (h two d) -> p h two d", two=2, d=D)[:, :, s, :]
                src = p.rearrange("p (h d) -> p h d", d=D)
                # all v drains on DVE: ACT is running head-0/1 exps by now
                nc.vector.tensor_copy(out=out3, in_=src)

            # (s, q) outer so stream 0's passes only need x0 (arrives
            # first); ft ordered so head 0/1's q AND k finish earliest --
            # the scheduler then starts head-0 scores/exp during the
            # qkv tail. Live psum banks: qk phase sp(4)+qk(4)=8; v phase
            # sp(4)+op2(2)+v(2)=8; ps_d reuses v's banks after release.
            for i in range(WARM_QK):
                pw = ps_qk.tile([128, 512], F32, tag="qkp", name="warm")
                nc.tensor.matmul(pw, lhsT=ones, rhs=warm,
                                 start=True, stop=True)
            for s in range(2):
                for q in range(NQ):
                    for ft in (0, 3, 1, 4, 2, 5):
                        qk_pass(ft, s, q)
            ps_qk.release()
            dsp = tc.alloc_tile_pool(name="dsp", bufs=2)
            ps_o = tc.alloc_tile_pool(name="ps_o", bufs=1, space="PSUM")
            ps_v = tc.alloc_tile_pool(name="ps_v", bufs=2, space="PSUM")
            # v passes for t>=2 are woven into head 0's loop below (PE
            # filler that keeps the tensor engine warm while the exp
            # stream paces); the debug cut emits them all here instead.
            if phase_cut == "qkv":
                for t in range(NK):
                    for s in range(2):
                        v_pass(s, t)
                ps_v.release()
                for h in range(HPC):
                    nc.sync.dma_start(out=yp[h * 128:(h + 1) * 128, 0:512],
                                      in_=qt[h][:, 0:512])
                    nc.sync.dma_start(out=yp[h * 128:(h + 1) * 128, 512:1024],
                                      in_=kt_[h][:, 0:512])
                for t in range(NK):
                    nc.sync.dma_start(
                        out=yp[C + (t % 6) * 128:C + (t % 6 + 1) * 128,
                               (t // 6) * 128:(t // 6) * 128 + 128],
                        in_=vt[t][:, 0:128])

            if phase_cut in (None, "attn"):
                # ---- phase 2: attention per head, sT[k, q] orientation ----
                # The exp stream on ACT is the pacer (8 x ~1.04us per head);
                # everything else is scheduled to never starve it:
                #  * scores for kb+2 are emitted ahead of all other PE work
                #    in each iteration (2-deep psum prefetch ring),
                #  * AV accumulation is rotated to start at kb5 and the
                #    kb0..4 products are deferred into the NEXT head's first
                #    two iterations, so the op2 psum buffer (single, PSUM is
                #    full) is reused only after the previous head's
                #    normalize has cleared -- no boundary stall,
                #  * the denominator uses a running sum (one DVE add after
                #    the last exp instead of a 3-level tree tail) and the
                #    finalize chain (ones-matmul -> reciprocal -> normalize,
                #    q-half split) is emitted at kb1 of the next head.
                ost = [[pp.tile([128, N], BF16, tag=f"ost{s}_{p}",
                                name=f"ost{s}_{p}")
                        for p in range(HPC // 2)] for s in range(2)]
                ps_d = None  # allocated at the h==1 boundary (reuses ps_v)

                AVS = 5  # first kb of the in-head AV accumulation window

                def scores_tile(h, kb, use_dp=False):
                    # use_dp: time-share the denominator psum banks for the
                    # next head's kb0 tile -- a 3rd effective prefetch slot
                    # that removes the head-boundary WAR stall on sp0.
                    if use_dp:
                        spt = ps_d.tile([128, N], F32, tag="dp",
                                        name="sp", bufs=1)
                    else:
                        spt = ps_s.tile([128, N], F32, tag=f"sp{kb % 2}",
                                        name="sp", bufs=1)
                    for q in range(NQ):
                        nc.tensor.matmul(
                            spt[:, q * 512:(q + 1) * 512],
                            lhsT=kt_[h][:, kb * 128:(kb + 1) * 128],
                            rhs=qt[h][:, q * 512:(q + 1) * 512],
                            start=True, stop=True)
                    return spt

                def av(h, kb, ex_t, op2_t):
                    for q in range(NQ):
                        nc.tensor.matmul(
                            op2_t[:, q * 512:(q + 1) * 512],
                            lhsT=vt[kb][:, h * 128:(h + 1) * 128],
                            rhs=ex_t[:, q * 512:(q + 1) * 512],
                            start=(kb == AVS), stop=(kb == AVS - 1))

                def fin_emit(h, st):
                    dp2 = ps_d.tile([128, N], F32, tag="dp",
                                    name="dp2", bufs=1)
                    rd = rp.tile([128, N], F32, tag="rd", name="rd")
                    for q in range(NQ):
                        ql = slice(q * 512, (q + 1) * 512)
                        nc.tensor.matmul(
                            dp2[:, ql], lhsT=ones, rhs=st["es"][:, ql],
                            start=True, stop=True)
                        nc.vector.reciprocal_approx_fast(
                            out=rd[:, ql], in_=dp2[:, ql])
                        for s in range(2):
                            nc.vector.tensor_mul(
                                out=ost[s][h // 2][(h % 2) * 64:
                                                   (h % 2) * 64 + 64, ql],
                                in0=st["op2"][s * 64:(s + 1) * 64, ql],
                                in1=rd[s * 64:(s + 1) * 64, ql])

                prev = None
                sp01 = [scores_tile(0, 0), scores_tile(0, 1)]
                for t in (0, 1):
                    for s in range(2):
                        v_pass(s, t)
                for h in range(HPC):
                    if h == 1:
                        # all v psum work was emitted in head 0's loop
                        ps_v.release()
                        ps_d = tc.alloc_tile_pool(name="ps_d", bufs=1,
                                                  space="PSUM")
                    op2 = ps_o.tile([128, N], F32, tag="op2", name="op2")
                    sp = [None] * NK
                    sp[0], sp[1] = sp01
                    ex = [None] * NK
                    run = None
                    es = None
                    for kb in range(NK):
                        ex[kb] = ep.tile([128, N], BF16, tag="exp", name="exp")
                        nc.scalar.activation(out=ex[kb], in_=sp[kb],
                                             func=AF.Exp)
                        if kb + 2 < NK:
                            sp[kb + 2] = scores_tile(h, kb + 2)
                        if h == 0 and kb + 2 < NK:
                            # weave the remaining v passes into head 0
                            v_pass(0, kb + 2)
                            v_pass(1, kb + 2)
                        if prev is not None:
                            if kb == 0:
                                for dkb in range(3):
                                    av(h - 1, dkb, prev["ex"][dkb],
                                       prev["op2"])
                            elif kb == 1:
                                for dkb in range(3, AVS):
                                    av(h - 1, dkb, prev["ex"][dkb],
                                       prev["op2"])
                                fin_emit(h - 1, prev)
                        if h + 1 < HPC:
                            # pre-emit next head's first score tiles ahead
                            # of this head's last AVs so exp(h+1,0) starts
                            # with zero bubble at the boundary; from h>=1
                            # kb0 goes through the dp banks (free until the
                            # next fin), which unpins the sp0 WAR entirely
                            if h >= 1 and kb == AVS:
                                sp01 = [scores_tile(h + 1, 0, use_dp=True),
                                        None]
                            elif h == 0 and kb == NK - 2:
                                sp01 = [scores_tile(h + 1, 0), None]
                            elif kb == NK - 1:
                                sp01[1] = scores_tile(h + 1, 1)
                        if kb >= AVS:
                            av(h, kb, ex[kb], op2)
                            if h == HPC - 1:
                                # last head: fold the deferred AVs into the
                                # in-head window so finalize fires at once
                                for dkb in range(2 * (kb - AVS),
                                                 min(2 * (kb - AVS) + 2, AVS)):
                                    av(h, dkb, ex[dkb], op2)
                        # running-sum denominator on DVE
                        if kb == 1:
                            run = dsp.tile([128, N], BF16, tag="run",
                                           name="run")
                            nc.vector.tensor_tensor(out=run, in0=ex[0],
                                                    in1=ex[1], op=AL.add)
                        elif 2 <= kb <= NK - 2:
                            nrun = dsp.tile([128, N], BF16, tag="run",
                                            name="run")
                            nc.vector.tensor_tensor(out=nrun, in0=run,
                                                    in1=ex[kb], op=AL.add)
                            run = nrun
                        else:
                            if kb == NK - 1:
                                es = dsp.tile([128, N], BF16, tag="es",
                                              name="es")
                                nc.vector.tensor_tensor(out=es, in0=run,
                                                        in1=ex[kb], op=AL.add)
                    prev = {"op2": op2, "ex": ex, "es": es}
                # last head's AVs were folded in-loop; just finalize
                fin_emit(HPC - 1, prev)
                ps_d.release()
                ps_o.release()
                dsp.release()
                ps_s.release()

                if phase_cut == "attn":
                    for s in range(2):
                        for p_ in range(HPC // 2):
                            nc.sync.dma_start(
                                out=yp[(s * 3 + p_) * 128:
                                       (s * 3 + p_ + 1) * 128, :],
                                in_=ost[s][p_])

            if phase_cut is None:
                # ---- phase 3: projection; partial y straight to DRAM ----
                # (cb,s) outer / q inner so both q-halves land in one yb
                # tile -> 12 full-row DMAs, split across two queues so
                # descriptor generation doesn't pace the tail.
                ps_y = tc.alloc_tile_pool(name="ps_y", bufs=6, space="PSUM")
                NP = HPC // 2
                # bridge the last finalize chain (~3.5us of serial DVE work)
                # with dummy matmuls so the PE clock stays hot into the proj
                for i in range(WARM_PROJ):
                    pw = ps_y.tile([128, 512], F32, tag="yp", name="warm")
                    nc.tensor.matmul(pw, lhsT=ones, rhs=warm,
                                     start=True, stop=True)
                for cb in range(C // 128):
                    for s in range(2):
                        yb = yp_pool.tile([128, N], BF16, tag="yb")
                        for q in range(NQ):
                            py = ps_y.tile([128, 512], F32, tag="yp",
                                           name="yp")
                            for p in range(NP):
                                nc.tensor.matmul(
                                    py,
                                    lhsT=wp_sb[:, p * C + cb * 128:
                                               p * C + (cb + 1) * 128],
                                    rhs=ost[s][p][:, q * 512:(q + 1) * 512],
                                    start=(p == 0), stop=(p == NP - 1))
                            if (q + s) % 2 == 0:
                                nc.scalar.activation(
                                    out=yb[:, q * 512:(q + 1) * 512],
                                    in_=py, func=AF.Copy)
                            else:
                                nc.vector.tensor_copy(
                                    out=yb[:, q * 512:(q + 1) * 512], in_=py)
                        eng = nc.sync if (cb + s) % 2 == 0 else nc.gpsimd
                        eng.dma_start(
                            out=yp[s * C + cb * 128:s * C + (cb + 1) * 128, :],
                            in_=yb)
                ps_y.release()

    nc.compile()
    return nc


_cache = threading.Lock()
_nc = None
_runner = None


def _get_program():
    global _nc
    with _cache:
        if _nc is None:
            _nc = build_program()
    return _nc


class _Runner:
    """Compile the 8-core sharded PJRT callable once and reuse it across
    kernel() calls (run_bass_kernel_spmd re-traces jax.jit per call, which
    costs seconds; the NEFF itself is what actually runs)."""

    def __init__(self, nc, n_cores=8):
        install_neuronx_cc_hook()
        self.nc = nc
        self.n_cores = n_cores
        partition_name = (nc.partition_id_tensor.name
                          if nc.partition_id_tensor else None)
        in_names, out_names, out_avals, zero_outs = [], [], [], []
        for alloc in nc.m.functions[0].allocations:
            if not isinstance(alloc, mybir.MemoryLocationSet):
                continue
            name = alloc.memorylocations[0].name
            if alloc.kind == "ExternalInput":
                if name != partition_name:
                    in_names.append(name)
            elif alloc.kind == "ExternalOutput":
                out_names.append(name)
                shape = tuple(alloc.tensor_shape)
                dtype = mybir.dt.np(alloc.dtype)
                out_avals.append(jax.core.ShapedArray(shape, dtype))
                zero_outs.append(
                    np.zeros((n_cores * shape[0], *shape[1:]), dtype))
        self.in_names = in_names
        self.out_names = out_names
        self.out_shapes = [tuple(a.shape) for a in out_avals]
        self.zero_outs = zero_outs
        n_params = len(in_names)
        n_outs = len(out_avals)
        all_in = list(in_names) + list(out_names)
        if partition_name is not None:
            all_in.append(partition_name)

        def _body(*args):
            operands = list(args)
            if partition_name is not None:
                operands.append(partition_id_tensor())
            outs = _bass_exec_p.bind(
                *operands,
                out_avals=tuple(out_avals),
                in_names=tuple(all_in),
                out_names=tuple(out_names),
                lowering_input_output_aliases=(),
                sim_require_finite=True,
                sim_require_nnan=True,
                nc=nc,
            )
            return tuple(outs)

        devices = jax.devices()[:n_cores]
        mesh = Mesh(np.asarray(devices), ("core",))
        self.f = jax.jit(
            shard_map(
                _body, mesh=mesh,
                in_specs=(PartitionSpec("core"),) * (n_params + n_outs),
                out_specs=(PartitionSpec("core"),) * n_outs,
                check_rep=False,
            ),
            keep_unused=True,
        )

    def run(self, in_maps):
        n = self.n_cores
        concat_in = [
            np.concatenate([np.asarray(in_maps[c][name]) for c in range(n)],
                           axis=0)
            for name in self.in_names
        ]
        out_arrs = self.f(*concat_in, *self.zero_outs)
        return [
            {name: np.asarray(out_arrs[i]).reshape(n, *self.out_shapes[i])[c]
             for i, name in enumerate(self.out_names)}
            for c in range(n)
        ]


def _get_runner():
    global _runner
    nc = _get_program()
    with _cache:
        if _runner is None:
            _runner = _Runner(nc)
    return _runner


def _f32_to_bf16(a):
    """Fast round-to-nearest f32->bf16 via integer ops (contiguous input)."""
    u = np.ascontiguousarray(a, np.float32).view(np.uint32)
    return (((u + 0x7FFF) + ((u >> 16) & 1)) >> 16).astype(np.uint16).view(BF)


def _bf16_to_f32(a):
    return (np.asarray(a).view(np.uint16).astype(np.uint32) << 16).view(
        np.float32)


_wprep_cache = {}


def _prep_weights(W_qkv, b_qkv, W_proj, b_proj):
    key = (id(W_qkv), id(b_qkv), id(W_proj), id(b_proj))
    hit = _wprep_cache.get(key)
    if hit is not None and (hit[0] is W_qkv and hit[1] is b_qkv
                            and hit[2] is W_proj and hit[3] is b_proj):
        return hit[4]
    W_qkv = np.asarray(W_qkv, np.float32)
    b_qkv = np.asarray(b_qkv, np.float32)
    W_proj = np.asarray(W_proj, np.float32)
    b_proj = np.asarray(b_proj, np.float32)
    Wq = W_qkv[0:C].reshape(H, D, C) * SCALE
    Wk = W_qkv[C:2 * C].reshape(H, D, C)
    Wv = W_qkv[2 * C:3 * C].reshape(H, D, C)
    bq = b_qkv[0:C].reshape(H, D) * SCALE
    bk = b_qkv[C:2 * C].reshape(H, D)
    bvv = b_qkv[2 * C:3 * C]

    per_group = []
    for g in range(2):
        hs = slice(g * HPC, (g + 1) * HPC)
        # wqk_sb[p, k*768 + ft*128 + hf*64 + d] =
        #   (Wq | Wk)[g*6 + 2*(ft%3) + hf, d, k*128 + p]
        wqk_cols = np.concatenate(
            [Wq[hs].reshape(HPC * D, C).T, Wk[hs].reshape(HPC * D, C).T],
            axis=1)                                        # [C, 768]
        wqk_sb = (wqk_cols.reshape(KT, 128, 2 * HPC * D)
                  .transpose(1, 0, 2).reshape(128, KT * C))
        # wv_sb[p, k*384 + h*64 + d] = Wv[g*6+h, d, k*128+p]
        wv_cols = Wv[hs].reshape(HPC * D, C).T             # [C, 384]
        wv_sb = (wv_cols.reshape(KT, 128, HPC * D)
                 .transpose(1, 0, 2).reshape(128, KT * HPC * D))
        # wp_sb[p, pq*768 + c] = W_proj[c, (g*6+2*pq)*64 + p]
        wproj = np.empty((HPC // 2, 128, C), np.float32)
        for p in range(HPC // 2):
            gh = g * HPC + 2 * p
            wproj[p, 0:64] = W_proj[:, gh * D:(gh + 1) * D].T
            wproj[p, 64:128] = W_proj[:, (gh + 1) * D:(gh + 2) * D].T
        wp_sb = wproj.reshape((HPC // 2) * 128, C).reshape(
            HPC // 2, 128, C).transpose(1, 0, 2).reshape(128, (HPC // 2) * C)
        # bq_sb[hf*64 + d, ft] = (bq | bk)[g*6 + 2*(ft%3) + hf, d]
        bq_sb = np.empty((128, HPC), np.float32)
        for ft in range(HPC):
            src = bq if ft < 3 else bk
            h0 = g * HPC + 2 * (ft % 3)
            bq_sb[0:64, ft] = src[h0]
            bq_sb[64:128, ft] = src[h0 + 1]
        per_group.append((
            np.ascontiguousarray(_f32_to_bf16(wqk_sb)),
            np.ascontiguousarray(_f32_to_bf16(wv_sb)),
            np.ascontiguousarray(_f32_to_bf16(wp_sb)),
            np.ascontiguousarray(bq_sb),
        ))
    # host-side constant: y += b_proj + W_proj @ b_v  (softmax rows sum to 1)
    y_const = b_proj + W_proj @ bvv                       # [C] f32
    prep = (per_group, y_const)
    _wprep_cache.clear()
    _wprep_cache[key] = (W_qkv, b_qkv, W_proj, b_proj, prep)
    return prep


def make_in_maps(x1, x2, W_qkv, b_qkv, W_proj, b_proj):
    """Host-side shard prep. Core c -> (batch c//2, head-group c%2)."""
    per_group, _ = _prep_weights(W_qkv, b_qkv, W_proj, b_proj)
    x1 = np.asarray(x1, np.float32)
    x2 = np.asarray(x2, np.float32)
    # x_sb[p, q*3072 + k*512 + t] = xT[k*128+p, q*512+t], bf16
    xs = []
    for x in (x1, x2):
        per_b = []
        for b in range(B):
            xt = _f32_to_bf16(np.ascontiguousarray(x[b].T))   # [768, 1024]
            per_b.append(np.ascontiguousarray(
                xt.reshape(KT, 128, NQ, 512).transpose(1, 2, 0, 3)
                .reshape(128, KT * N)))
            # note axis order (p, q, k, t): col = q*(KT*512) + k*512 + t
        xs.append(per_b)

    in_maps = []
    for c in range(8):
        b, g = divmod(c, 2)
        wqk_sb, wv_sb, wp_sb, bq_sb = per_group[g]
        in_maps.append({
            "wqk": wqk_sb,
            "wv": wv_sb,
            "wp": wp_sb,
            "x0": xs[0][b],
            "x1": xs[1][b],
            "bq": bq_sb,
        })
    return in_maps


def combine_outputs(results, y_const):
    y1 = np.empty((B, N, C), np.float32)
    y2 = np.empty((B, N, C), np.float32)
    for b in range(B):
        p0 = _bf16_to_f32(results[2 * b]["yp"])
        p1 = _bf16_to_f32(results[2 * b + 1]["yp"])
        ysum = p0 + p1                                     # [2C, N]
        y1[b] = ysum[0:C].T + y_const
        y2[b] = ysum[C:2 * C].T + y_const
    return y1, y2


def kernel(x1, x2, W_qkv, b_qkv, W_proj, b_proj):
    in_maps = make_in_maps(x1, x2, W_qkv, b_qkv, W_proj, b_proj)
    _, y_const = _prep_weights(W_qkv, b_qkv, W_proj, b_proj)
    try:
        results = _get_runner().run(in_maps)
    except Exception:
        # robust fallback: the one-shot path run_bass_kernel_spmd uses
        nc = _get_program()
        results = run_bass_kernel_spmd(
            nc, in_maps, core_ids=list(range(8))).results
    return combine_outputs(results, y_const)
